# revision 6
# baseline (speedup 1.0000x reference)
"""Trainium2 Bass kernel for nn_DeformKernelConv2d.

Math (per batch image; shapes below are per core after sharding):
  offsets:  off = conv3x3(x, offset_w) + offset_b          -> dy,dx per (k, pixel)
  coords:   yc_k = dy_k + by_k ; xc_k = dx_k + bx_k        (scope-kernel space)
  phi:      phi_y[k,i] = relu(1-|yc_k - i|), i=0..3        (likewise phi_x)
  Phi:      Phi_k[4*yi+xi] = phi_y[k,yi] * phi_x[k,xi]     (bilinear weights, 16 per k)
  samp:     samp_k[c] = sum_s wflat[c,s] * Phi_k[s]        (matmul over s=16)
  out:      out[c] = sum_k samp_k[c] * x_k[c]              (x_k = 3x3-shifted x)

Device mapping:
  - 8 cores: (batch b, H-half); each core does 28 rows with a 1-row halo.
  - The offset conv is fused with the affine (coord - grid line i) expansion
    into one 9-tap accumulated matmul producing T[72, pix] (rows (k, axis, i)),
    with per-row bias = offset_b + base - i folded into the Abs activation.
  - phi on ScalarE (Abs then Relu).
  - phi -> Y/X row replication done ON THE PE with 0/1 replication matrices
    (6 small matmuls per chunk) instead of 72 strided SBUF->SBUF DMAs.
  - x loaded once as fp32 via HWDGE, cast to the two bf16 shifted copies
    on ScalarE/VectorE (the SWDGE cast-DMA path is ~10x slower).
  - Phi products + final MAC split across VectorE and GpSimdE.
"""

import numpy as np
import ml_dtypes

B, C, H, W = 4, 128, 56, 56
HC = H // 2            # 28 rows per core
NPIX = HC * W          # 1568
CH = 7                 # chunk height (rows)
NCH = HC // CH         # 4 chunks
CHN = CH * W           # 392 columns per chunk
RA, RB = 58, 60        # padded row lengths: xbfA data at col 1, xbfB at col 2

_BF16 = ml_dtypes.bfloat16
_cache = {}


def _build_program():
    import concourse.tile as tile
    import concourse.mybir as mybir
    from concourse import bacc

    fp32 = mybir.dt.float32
    bf16 = mybir.dt.bfloat16
    AF = mybir.ActivationFunctionType

    nc = bacc.Bacc("TRN2", target_bir_lowering=False, debug=False, num_devices=8)
    xs_d = nc.dram_tensor("xs", [C, HC + 2, W], fp32, kind="ExternalInput")
    lhsT_d = nc.dram_tensor("lhsT", [C, 9 * 72], bf16, kind="ExternalInput")
    w4T_d = nc.dram_tensor("w4T", [C, C], bf16, kind="ExternalInput")
    bias_d = nc.dram_tensor("bias72", [72, 1], fp32, kind="ExternalInput")
    rall_d = nc.dram_tensor("rall", [72, 544], bf16, kind="ExternalInput")
    out_d = nc.dram_tensor("out", [C, HC, W], fp32, kind="ExternalOutput")

    with tile.TileContext(nc) as tc:
        with (
            tc.tile_pool(name="const", bufs=1) as cp,
            tc.tile_pool(name="work", bufs=1) as wp,
            tc.tile_pool(name="tmp", bufs=2) as tp,
            tc.tile_pool(name="psT", bufs=2, space="PSUM") as ppT,
            tc.tile_pool(name="psR", bufs=3, space="PSUM") as ppR,
            tc.tile_pool(name="psS", bufs=3, space="PSUM") as ppS,
        ):
            lhsT = cp.tile([C, 9 * 72], bf16)
            nc.sync.dma_start(lhsT[:], lhsT_d[:])
            w4T = cp.tile([C, C], bf16)
            nc.sync.dma_start(w4T[:], w4T_d[:])
            bias = cp.tile([72, 1], fp32)
            nc.sync.dma_start(bias[:], bias_d[:])
            rall = cp.tile([72, 544], bf16)
            nc.sync.dma_start(rall[:], rall_d[:])
            # replication matrix views
            RYA = rall[:, 0:128]
            RXA = rall[:, 128:256]
            RYB = rall[:, 256:384]
            RXB = rall[:, 384:512]
            RYC = rall[:, 512:528]
            RXC = rall[:, 528:544]

            xf32 = cp.tile([C, HC + 2, W], fp32)
            nc.sync.dma_start(xf32[:], xs_d[:])

            xbfA = cp.tile([C, HC + 2, RA], bf16)
            xbfB = cp.tile([C, HC + 2, RB], bf16)
            nc.vector.memset(xbfA[:, :, 0:1], 0)
            nc.vector.memset(xbfA[:, :, 57:58], 0)
            nc.vector.memset(xbfB[:, :, 0:2], 0)
            nc.vector.memset(xbfB[:, :, 58:60], 0)
            # fp32 -> bf16 shifted copies (on-chip cast; no SWDGE cast-DMA)
            nc.scalar.copy(xbfA[:, :, 1:57], xf32[:])
            nc.vector.tensor_copy(out=xbfB[:, :, 2:58], in_=xf32[:])

            HH = NPIX // 2  # 784 pixels (14 rows) per final-stage half
            samp = wp.tile([C, 9, NPIX], bf16)
            prod = wp.tile([C, 2, 9, HH], bf16)
            t1 = wp.tile([C, 2, 4, HH], bf16)
            t2 = wp.tile([C, 2, 2, HH], bf16)
            t3 = wp.tile([C, 2, HH], bf16)
            res = wp.tile([C, 2, HH], fp32)

            for ch in range(NCH):
                # ---- T matmul (offset conv + affine expansion) ----
                psT = ppT.tile([72, CHN], fp32, tag="psT")
                for tap in range(9):
                    di, dj = tap // 3, tap % 3
                    rhs = xbfA[:, ch * CH + di : ch * CH + di + CH, dj : dj + W]
                    nc.tensor.matmul(
                        psT[:],
                        lhsT[:, tap * 72 : (tap + 1) * 72],
                        rhs,
                        start=(tap == 0),
                        stop=(tap == 8),
                    )
                u = tp.tile([72, CHN], fp32, tag="u")
                nc.scalar.activation(u[:], psT[:], AF.Abs, bias=bias[:], scale=1.0)
                phi = tp.tile([72, CHN], bf16, tag="phi")
                nc.scalar.activation(phi[:], u[:], AF.Relu, bias=1.0, scale=-1.0)

                # ---- replication matmuls + Phi products ----
                psYA = ppR.tile([C, CHN], fp32, tag="psR")
                nc.tensor.matmul(psYA[:], RYA, phi[:], start=True, stop=True)
                YA = tp.tile([C, CHN], bf16, tag="YA")
                nc.scalar.copy(YA[:], psYA[:])
                psXA = ppR.tile([C, CHN], fp32, tag="psR")
                nc.tensor.matmul(psXA[:], RXA, phi[:], start=True, stop=True)
                PhA = tp.tile([C, CHN], bf16, tag="PhA")
                nc.vector.tensor_mul(PhA[:], YA[:], psXA[:])

                psYB = ppR.tile([C, CHN], fp32, tag="psR")
                nc.tensor.matmul(psYB[:], RYB, phi[:], start=True, stop=True)
                YB = tp.tile([C, CHN], bf16, tag="YB")
                nc.scalar.copy(YB[:], psYB[:])
                psXB = ppR.tile([C, CHN], fp32, tag="psR")
                nc.tensor.matmul(psXB[:], RXB, phi[:], start=True, stop=True)
                PhB = tp.tile([C, CHN], bf16, tag="PhB")
                nc.vector.tensor_mul(PhB[:], YB[:], psXB[:])

                psYC = ppR.tile([16, CHN], fp32, tag="psR")
                nc.tensor.matmul(psYC[:], RYC, phi[:], start=True, stop=True)
                YC = tp.tile([16, CHN], bf16, tag="YC")
                nc.scalar.copy(YC[:], psYC[:])
                psXC = ppR.tile([16, CHN], fp32, tag="psR")
                nc.tensor.matmul(psXC[:], RXC, phi[:], start=True, stop=True)
                PhC = tp.tile([16, CHN], bf16, tag="PhC")
                nc.vector.tensor_mul(PhC[:], YC[:], psXC[:])

                # ---- samp matmuls (row-packed), PSUM drain ----
                for k in range(9):
                    g = k % 4
                    src = (PhA, PhB, PhC)[k // 4]
                    base = 32 * g if k < 8 else 0
                    psS = ppS.tile([C, CHN], fp32, tag="psS")
                    nc.tensor.matmul(
                        psS[:],
                        w4T[base : base + 16, :],
                        src[base : base + 16, :],
                        start=True,
                        stop=True,
                        tile_position=(base, 0),
                    )
                    dst = samp[:, k, ch * CHN : (ch + 1) * CHN]
                    if k < 5:
                        nc.scalar.copy(dst, psS[:])
                    else:
                        nc.vector.tensor_copy(out=dst, in_=psS[:])

                # ---- final stage per half: emitted one chunk late so the
                # next chunk's PE work is already queued (HAM stays warm) ----
                for h in [0] if ch == 2 else ([1] if ch == 3 else []):
                    HR = HC // 2  # 14 rows per half
                    for k in range(9):
                        di, dj = k // 3, k % 3
                        if dj == 1:
                            xsrc, coff = xbfB, dj + 1  # col offset 2: 4B aligned
                        else:
                            xsrc, coff = xbfA, dj  # col offsets 0, 2
                        xv = xsrc[:, h * HR + di : h * HR + di + HR, coff : coff + W]
                        nc.vector.tensor_mul(
                            prod[:, h, k, :].rearrange("p (h w) -> p h w", h=HR),
                            samp[:, k, h * HH : (h + 1) * HH].rearrange(
                                "p (h w) -> p h w", h=HR
                            ),
                            xv,
                        )
                    ph = prod[:, h]
                    nc.vector.tensor_add(
                        t1[:, h].rearrange("p a n -> p (a n)"),
                        ph[:, 0:4].rearrange("p a n -> p (a n)"),
                        ph[:, 4:8].rearrange("p a n -> p (a n)"),
                    )
                    nc.vector.tensor_add(
                        t2[:, h].rearrange("p a n -> p (a n)"),
                        t1[:, h, 0:2].rearrange("p a n -> p (a n)"),
                        t1[:, h, 2:4].rearrange("p a n -> p (a n)"),
                    )
                    nc.vector.tensor_add(t3[:, h], t2[:, h, 0], t2[:, h, 1])
                    nc.vector.tensor_add(res[:, h], t3[:, h], ph[:, 8])
                    nc.sync.dma_start(
                        out_d[:, h * HR : (h + 1) * HR, :],
                        res[:, h].rearrange("p (h w) -> p h w", h=HR),
                    )

    nc.finalize()
    return nc


def _prep_inputs(x, offset_w, offset_b, weight):
    """Host-side sharding + weight reshaping. Returns per-core input maps."""
    x = np.asarray(x, dtype=np.float32)
    offset_w = np.asarray(offset_w, dtype=np.float32)
    offset_b = np.asarray(offset_b, dtype=np.float32)
    weight = np.asarray(weight, dtype=np.float32)

    # lhsT[c, tap*72 + k*8 + axis*4 + i] = offset_w[2k+axis, c, tap//3, tap%3]
    ow = offset_w.reshape(9, 2, C, 3, 3)  # [k, axis, c, di, dj]
    lhsT = np.transpose(ow, (2, 3, 4, 0, 1))  # [c, di, dj, k, axis]
    lhsT = np.repeat(lhsT[..., None], 4, axis=-1)  # [c, di, dj, k, axis, i]
    lhsT = np.ascontiguousarray(lhsT.reshape(C, 648)).astype(_BF16)

    # w4T rows 32g+s = weight[:, s//4, s%4]
    w4T = np.zeros((C, C), dtype=_BF16)
    wT = weight.reshape(C, 16).T.astype(_BF16)  # [16, C]
    for g in range(4):
        w4T[32 * g : 32 * g + 16, :] = wT

    # bias72[k*8+axis*4+i] = offset_b[2k+axis] + base - i
    base = np.arange(3, dtype=np.float32) + 0.5
    bias = np.zeros((9, 2, 4), dtype=np.float32)
    for k in range(9):
        for axis in range(2):
            bv = base[k // 3] if axis == 0 else base[k % 3]
            bias[k, axis, :] = offset_b[2 * k + axis] + bv - np.arange(4)
    bias72 = bias.reshape(72, 1)

    # replication matrices: phi row (k*8 + a*4 + i) -> Y/X rows (32g + yi*4+xi)
    rall = np.zeros((72, 544), dtype=_BF16)
    for k in range(9):
        for yi in range(4):
            for xi in range(4):
                s = yi * 4 + xi
                ry = k * 8 + yi
                rx = k * 8 + 4 + xi
                if k < 4:
                    rall[ry, 0 + 32 * k + s] = 1
                    rall[rx, 128 + 32 * k + s] = 1
                elif k < 8:
                    rall[ry, 256 + 32 * (k - 4) + s] = 1
                    rall[rx, 384 + 32 * (k - 4) + s] = 1
                else:
                    rall[ry, 512 + s] = 1
                    rall[rx, 528 + s] = 1

    in_maps = []
    for core in range(8):
        b, half = core // 2, core % 2
        h0 = half * HC
        xs = np.zeros((C, HC + 2, W), dtype=np.float32)
        lo, hi = h0 - 1, h0 + HC + 1
        slo, shi = max(lo, 0), min(hi, H)
        xs[:, slo - lo : slo - lo + (shi - slo), :] = x[b, :, slo:shi, :]
        in_maps.append(
            {"xs": xs, "lhsT": lhsT, "w4T": w4T, "bias72": bias72, "rall": rall}
        )
    return in_maps


def kernel(x, offset_w, offset_b, weight):
    from concourse.bass_utils import run_bass_kernel_spmd

    if "nc" not in _cache:
        _cache["nc"] = _build_program()
    nc = _cache["nc"]

    in_maps = _prep_inputs(x, offset_w, offset_b, weight)
    res = run_bass_kernel_spmd(nc, in_maps, core_ids=list(range(8)))

    out = np.zeros((B, C, H, W), dtype=np.float32)
    for core in range(8):
        b, half = core // 2, core % 2
        out[b, :, half * HC : (half + 1) * HC, :] = res.results[core]["out"].reshape(
            C, HC, W
        )
    return out
